# revision 2
# baseline (speedup 1.0000x reference)
"""CNOT-ring permutation kernel for Trainium2 (Bass, 8 NeuronCores) — v2.

Problem: state [32, 2^20, 2] f32; ring of CNOTs CNOT(i, (i+1)%20) composes to

    out[b, y, :] = in[b, x(y), :],   x(y) = (y ^ (y>>1)) ^ ((y&1) * (3<<18))

Data-parallel over batch (4 rows/core).  Per-core, per row: 1024 output
blocks of 1024 amps; output block bp gathers from input blocks gray(bp)
and gray(bp)^768 (even/odd amp split); bp and bp^512 share the same input
pair, so one partition builds both (exact HBM traffic, 64 MiB/core).

v2 changes vs v1 (220 us):
  - 2 SBUF buffer sets instead of 3 (pass B writes back into tin), NBUF=6
    instead of 4 -> much deeper pipeline, DMA engines stay fed.
  - pass A: 24 pieces (double merge {7,9} on the c5=0 half) vs 32.
  - pass B split DVE/ACT (merged 5-dim pieces on DVE, 4-dim on ACT),
    balancing both engines at ~6.7 us/tile, ~40% slack under the
    ~11.6 us/tile DMA floor.
Free-dim map (src i as fn of dst o, 12 bits):
    i_k = o_k ^ o_{k+1} (k=1..9), i11 = o1 ^ o11, i0 = o0, i10 = o10
Factored F = A∘B with A = {(k,k+1) k=4..9}, B = {(1,2),(2,3),(3,4),(11,1)}
(controls(A) ∩ targets(B) = ∅ so the 2-pass composition is exact).
"""

from contextlib import ExitStack
from itertools import product

import numpy as np

ROWS_PER_CORE = 4
N_CORES = 8
NAMP = 1 << 20
ROW_F32 = NAMP * 2
NBLK = 1024
BLK_F32 = 2048
HALF_F32 = 1024
TILES_PER_ROW = 4
NF = 4096
NBUF = 6

PASS_A = [(k, k + 1) for k in range(4, 10)]               # targets 4..9
PASS_B = [(1, 2), (2, 3), (3, 4), (11, 1)]                # targets 1,2,3,11
# pass A piece plan: [(extra_fixed {bit: val}, fixed_bits, merge_bits)]
A_PLAN = [
    ({5: 0}, [6, 8, 10], [7, 9]),   # 8 pieces,  FD=256
    ({5: 1}, [6, 8, 9, 10], [7]),   # 16 pieces, FD=128
]
N_B_DVE = 2   # merged B pieces (of 8) run on DVE; ACT runs the rest unmerged


def _gray(v):
    return v ^ (v >> 1)


def make_gather_idxs(rows=ROWS_PER_CORE):
    """int16 index tensor for dma_gather, identical to v1."""
    cols = []
    for t in range(TILES_PER_ROW):
        idxs = np.zeros((4, 128), np.int16)
        for p in range(128):
            bp = t * 128 + p
            X = _gray(bp)
            XC = X ^ 768
            sw = bp & 1
            idxs[0, p] = 2 * X + sw
            idxs[1, p] = 2 * X + (1 - sw)
            idxs[2, p] = 2 * XC + sw
            idxs[3, p] = 2 * XC + (1 - sw)
        flat = idxs.reshape(-1)
        ncol = len(flat) // 16
        wrapped = flat.reshape(ncol, 16).T
        cols.append(np.tile(wrapped, (8, 1)))
    return np.concatenate(cols, axis=1)


def _src_of(o, tcs):
    m = 0
    for t, c in tcs:
        m ^= ((o >> c) & 1) << t
    return o ^ m


def xor_pieces(tcs, branches, nbits=12):
    """Abstract pieces for a simultaneous XOR-class pass.

    branches: list of (pre_fixed {bit: val}, fixed_bits, merge_bits).
    Returns list of (dst_offset, dims_dst, src_offset, dims_src), dims as
    [stride, count] WITHOUT the partition dim.  Brute-force-verified affine.
    """
    controls = sorted({c for _, c in tcs})
    targets = {t for t, _ in tcs}
    out = []
    for pre, fixed, merge in branches:
        assert set(pre) | set(fixed) | set(merge) == set(controls)
        for m in merge:
            # merged control's target must sit on a fixed/pre-fixed position
            mt = [t for t, c in tcs if c == m]
            for t in mt:
                assert t in set(pre) | set(fixed) | set(merge), (m, t)
        free = [b for b in range(nbits)
                if b not in controls and b not in merge]
        for vals in product([0, 1], repeat=len(fixed)):
            cval = dict(pre)
            cval.update(dict(zip(fixed, vals)))
            base = 0
            for c, v in cval.items():
                base |= v << c
            mask_fixed = 0
            for t, c in tcs:
                if c in cval:
                    mask_fixed ^= cval[c] << t
            flips = [t for t in free if (mask_fixed >> t) & 1 and t in targets]
            # dst dims: merges (desc), then free runs split at flip bits
            dims = [(1 << m2, 2, False) for m2 in sorted(merge, reverse=True)]
            run = []
            for b in sorted(free, reverse=True):
                if b in flips:
                    if run:
                        dims.append((1 << run[-1], 1 << len(run), False))
                        run = []
                    dims.append((1 << b, 2, True))
                else:
                    if run and run[-1] != b + 1:
                        dims.append((1 << run[-1], 1 << len(run), False))
                        run = []
                    run.append(b)
            if run:
                dims.append((1 << run[-1], 1 << len(run), False))
            # src offset/strides from corner differences
            def pos(idxs):
                o = base
                for (st, n, _), i in zip(dims, idxs):
                    o += st * i
                return o
            corner = [0] * len(dims)
            s0 = _src_of(pos(corner), tcs)
            sstr = []
            for d in range(len(dims)):
                step = list(corner)
                step[d] = 1
                sstr.append(_src_of(pos(step), tcs) - s0)
            # brute-force affine check over the full piece
            for idxs in product(*[range(n) for _, n, _ in dims]):
                want = _src_of(pos(list(idxs)), tcs)
                got = s0 + sum(ss * i for ss, i in zip(sstr, idxs))
                assert want == got, (cval, idxs, want, got)
            dims_dst = [[st, n] for st, n, _ in dims]
            dims_src = [[ss, n] for ss, (_, n, _) in zip(sstr, dims)]
            out.append((base, dims_dst, s0, dims_src))
    return out


def b_piece_split():
    """Pass B pieces: merged (5-dim) combos for DVE + unmerged (4-dim) for ACT.

    Returns (dve_pieces, act_pieces) as abstract pieces."""
    merged = xor_pieces(PASS_B, [({}, [1, 3, 4], [2])])
    assert len(merged) == 8
    dve = merged[:N_B_DVE]
    # ACT takes the complementary combos unmerged.  merged piece i covers
    # (c1,c3,c4) combo i (order of product over fixed [1,3,4]).
    unmerged = xor_pieces(PASS_B, [({}, [1, 2, 3, 4], [])])
    assert len(unmerged) == 16
    # map each unmerged piece to its (c1,c3,c4) combo index
    def combo_of(base):
        c1, c3, c4 = (base >> 1) & 1, (base >> 3) & 1, (base >> 4) & 1
        # product order over fixed [1,3,4]: c1 slowest
        return (c1 << 2) | (c3 << 1) | c4
    taken = {combo_of(m[0]) for m in dve}
    act = [u for u in unmerged if combo_of(u[0]) not in taken]
    assert len(act) == 16 - 2 * N_B_DVE
    return dve, act


A_PIECES = xor_pieces(PASS_A, A_PLAN)
B_DVE, B_ACT = b_piece_split()


def apply_pieces_np(src_arr, dst_arr, pieces):
    """Numpy model: dst[o] = src[i] per abstract piece list (1 partition)."""
    for dbase, ddims, sbase, sdims in pieces:
        for idxs in product(*[range(n) for _, n in ddims]):
            d = dbase + sum(st * i for (st, _), i in zip(ddims, idxs))
            s = sbase + sum(st * i for (st, _), i in zip(sdims, idxs))
            dst_arr[d] = src_arr[s]


def validate_pieces():
    """Check A then B pieces implement the documented free-dim map."""
    rng = np.random.default_rng(0)
    tin = rng.integers(0, 1 << 30, NF).astype(np.int64)
    tmid = np.full(NF, -1, np.int64)
    tout = np.full(NF, -1, np.int64)
    apply_pieces_np(tin, tmid, A_PIECES)
    apply_pieces_np(tmid, tout, B_DVE + B_ACT)
    tcs_full = [(k, k + 1) for k in range(1, 10)] + [(11, 1)]
    exp = np.empty(NF, np.int64)
    for o in range(NF):
        exp[o] = tin[_src_of(o, tcs_full)]
    assert np.array_equal(tout, exp), "piece validation FAILED"
    assert (tmid != -1).all() and (tout != -1).all()
    return True


def validate_end_to_end():
    """Full numpy model of one row: gather + passes + stores vs closed form."""
    rng = np.random.default_rng(1)
    x = rng.integers(0, 1 << 30, ROW_F32).astype(np.int64)
    xv = x.reshape(2048, HALF_F32)
    y = np.full(ROW_F32, -1, np.int64)
    yv = y.reshape(NBLK, BLK_F32)
    idx = make_gather_idxs()
    for t in range(TILES_PER_ROW):
        table = idx[:16, t * 32:(t + 1) * 32]        # [16, 32]
        flat = table.T.reshape(-1)                    # j*128+p order
        tin = np.empty((128, NF), np.int64)
        for j in range(4):
            for p in range(128):
                tin[p, j * 1024:(j + 1) * 1024] = xv[flat[j * 128 + p]]
        tmid = np.empty_like(tin)
        tfin = np.empty_like(tin)
        for p in range(128):
            apply_pieces_np(tin[p], tmid[p], A_PIECES)
            apply_pieces_np(tmid[p], tfin[p], B_DVE + B_ACT)
        yv[t * 128:(t + 1) * 128] = tfin[:, 0:BLK_F32]
        yv[512 + t * 128: 512 + (t + 1) * 128] = tfin[:, BLK_F32:NF]
    yy = np.arange(NAMP)
    xx = (yy ^ (yy >> 1)) ^ ((yy & 1) * (3 << 18))
    exp = x.reshape(NAMP, 2)[xx].reshape(-1)
    assert np.array_equal(y, exp), "end-to-end validation FAILED"
    return True


def build_kernel(rows=ROWS_PER_CORE):
    """Per-core Bass program.  Inputs: x [rows, ROW_F32] f32, idx [128,128]
    int16.  Output: y [rows, ROW_F32] f32."""
    import concourse.bacc as bacc
    import concourse.mybir as mybir
    from concourse.ap import AP
    from concourse.library_config import mlp

    nc = bacc.Bacc("TRN2", target_bir_lowering=False, debug=False)
    x = nc.dram_tensor("x", [rows, ROW_F32], mybir.dt.float32, kind="ExternalInput")
    idx = nc.dram_tensor("idx", [128, 128], mybir.dt.int16, kind="ExternalInput")
    y = nc.dram_tensor("y", [rows, ROW_F32], mybir.dt.float32, kind="ExternalOutput")

    ntiles = rows * TILES_PER_ROW

    with (
        nc.sbuf_tensor("tidx", [128, 128], mybir.dt.int16) as tidx,
        nc.semaphore("s_idx") as s_idx,
        nc.semaphore("s_A") as s_A,       # DVE pass-A completions (per tile)
        nc.semaphore("s_Bd") as s_Bd,     # DVE pass-B completions
        nc.semaphore("s_Ba") as s_Ba,     # ACT pass-B completions
        ExitStack() as stack,
        nc.Block() as block,
    ):
        tin = [stack.enter_context(nc.sbuf_tensor(f"tin{b}", [128, NF], mybir.dt.float32)) for b in range(NBUF)]  # noqa: ANT232
        tmid = [stack.enter_context(nc.sbuf_tensor(f"tmid{b}", [128, NF], mybir.dt.float32)) for b in range(NBUF)]  # noqa: ANT232
        s_load = [stack.enter_context(nc.semaphore(f"s_load{b}")) for b in range(NBUF)]  # noqa: ANT232
        s_store = [stack.enter_context(nc.semaphore(f"s_store{b}")) for b in range(NBUF)]  # noqa: ANT232

        def mk_aps(tile_in, tile_out, pieces):
            pstride = tile_in.ap().ap[0][0]
            out = []
            for dbase, ddims, sbase, sdims in pieces:
                out.append((
                    AP(tensor=tile_out.ap().tensor, offset=dbase,
                       ap=[[pstride, 128]] + ddims),
                    AP(tensor=tile_in.ap().tensor, offset=sbase,
                       ap=[[pstride, 128]] + sdims),
                ))
            return out

        a_aps = [mk_aps(tin[b], tmid[b], A_PIECES) for b in range(NBUF)]
        bd_aps = [mk_aps(tmid[b], tin[b], B_DVE) for b in range(NBUF)]
        ba_aps = [mk_aps(tmid[b], tin[b], B_ACT) for b in range(NBUF)]

        xv = x.rearrange("r (n e) -> r n e", e=HALF_F32)   # [rows, 2048, 1024]
        yv = y.rearrange("r (n e) -> r n e", e=BLK_F32)    # [rows, 1024, 2048]

        @block.gpsimd
        def _(g):
            g.load_library(mlp)
            g.wait_ge(s_idx, 16)
            for i in range(ntiles):
                r, t = divmod(i, TILES_PER_ROW)
                b = i % NBUF
                if i >= NBUF:
                    # tin[b] free once tile i-NBUF's stores are done
                    g.wait_ge(s_store[b], 32 * (i // NBUF))
                g.dma_gather(
                    tin[b][:, :].rearrange("p (j e) -> p j e", e=HALF_F32),
                    xv[r],
                    tidx[:, t * 32:(t + 1) * 32],
                    512, 512, HALF_F32,
                ).then_inc(s_load[b], 16)

        @block.vector
        def _(v):
            for i in range(ntiles):
                b = i % NBUF
                v.wait_ge(s_load[b], 16 * (i // NBUF + 1))
                if i >= NBUF:
                    # tmid[b] free once B pieces of tile i-NBUF are done
                    # (s_Bd self-wait orders vs DVE's own datapath)
                    v.wait_ge(s_Ba, i - NBUF + 1)
                    v.wait_ge(s_Bd, i - NBUF + 1)
                aps = a_aps[b]
                for n, (dst, src) in enumerate(aps):
                    ins = v.tensor_copy(dst, src)
                    if n == len(aps) - 1:
                        ins.then_inc(s_A, 1)
                # self-wait: order B reads after the pass-A datapath drains
                v.wait_ge(s_A, i + 1)
                for n, (dst, src) in enumerate(bd_aps[b]):
                    ins = v.tensor_copy(dst, src)
                    if n == len(bd_aps[b]) - 1:
                        ins.then_inc(s_Bd, 1)

        @block.scalar
        def _(s):
            for i in range(ntiles):
                b = i % NBUF
                s.wait_ge(s_A, i + 1)
                aps = ba_aps[b]
                for n, (dst, src) in enumerate(aps):
                    ins = s.copy(dst, src)
                    if n == len(aps) - 1:
                        ins.then_inc(s_Ba, 1)

        @block.sync
        def _(sy):
            sy.dma_start(tidx[:, :], idx[:, :]).then_inc(s_idx, 16)
            for i in range(ntiles):
                r, t = divmod(i, TILES_PER_ROW)
                b = i % NBUF
                sy.wait_ge(s_Bd, i + 1)
                sy.wait_ge(s_Ba, i + 1)
                sy.dma_start(
                    yv[r, t * 128:(t + 1) * 128, :], tin[b][:, 0:BLK_F32]
                ).then_inc(s_store[b], 16)
                sy.dma_start(
                    yv[r, 512 + t * 128: 512 + (t + 1) * 128, :], tin[b][:, BLK_F32:NF]
                ).then_inc(s_store[b], 16)
            for b in range(NBUF):
                n_b = len([i for i in range(ntiles) if i % NBUF == b])
                sy.wait_ge(s_store[b], 32 * n_b)

    nc.compile()
    return nc


_IDX = None
_NC = None


def kernel(state: np.ndarray) -> np.ndarray:
    """Full-input entry point: state [32, 2^20, 2] f32 -> same shape."""
    global _IDX, _NC
    from concourse.bass_utils import run_bass_kernel_spmd

    assert state.shape == (32, NAMP, 2) and state.dtype == np.float32
    if _IDX is None:
        _IDX = make_gather_idxs()
    if _NC is None:
        _NC = build_kernel(ROWS_PER_CORE)

    in_maps = []
    for c in range(N_CORES):
        xs = np.ascontiguousarray(
            state[c * ROWS_PER_CORE:(c + 1) * ROWS_PER_CORE]
        ).reshape(ROWS_PER_CORE, ROW_F32)
        in_maps.append({"x": xs, "idx": _IDX})

    res = run_bass_kernel_spmd(_NC, in_maps, core_ids=list(range(N_CORES)))
    out = np.empty((32, NAMP, 2), np.float32)
    for c in range(N_CORES):
        out[c * ROWS_PER_CORE:(c + 1) * ROWS_PER_CORE] = res.results[c]["y"].reshape(
            ROWS_PER_CORE, NAMP, 2
        )
    return out


if __name__ == "__main__":
    print("pieces: A", len(A_PIECES), "B_DVE", len(B_DVE), "B_ACT", len(B_ACT))
    print("validate_pieces:", validate_pieces())
    print("validate_end_to_end:", validate_end_to_end())
